# revision 1
# baseline (speedup 1.0000x reference)
"""Trainium2 Bass kernel for CapsNet dynamic routing (nn_CapsRoutingLayer).

Reference computation (see problem):
    x_hat[b,i,o,d] = sum_k W[i,o,d,k] * x[b,i,k]
    b_logits = 0
    for it in 0..2:
        c = softmax_o(b_logits); s[b,o,d] = sum_i c[b,i,o] x_hat[b,i,o,d]
        v = squash(s)   # global Frobenius norm over whole s tensor
        if it < 2: b_logits += sum_d x_hat[b,i,o,d] v[b,o,d]
    return v  # (128, 32, 32)

Sharding: input capsules i (1152) split across 8 cores (144 each). W shard
(18.9MB fp32) stays resident in SBUF; x_hat is regenerated on the fly each
routing iteration via PE matmuls (4x row-tiled, K=32) and consumed from PSUM
by DVE ops, so the 604MB x_hat tensor never exists in full. The per-iteration
partial s (and the squash norm) are combined across cores with an on-device
AllReduce of the tiny [128,1024] s tensor.

Host-side prep: per-core shards of W and x are pre-transposed with numpy into
the SBUF layouts the kernel wants:
    ws[(g,k), j, (o,d)] = W[i0 + 4j+g, o, d, k]     (128, 36, 1024) fp32
    xs[(g,k), j, b]     = x[b, i0 + 4j+g, k]        (128, 36, 128)  fp32
"""

import numpy as np

from concourse import bacc, bass_isa, bass_utils, mybir, tile

N_CORES = 8
B = 128          # batch
NI = 1152        # input capsules
K = 32           # dim_input
NO = 32          # output capsules
D = 32           # dim_output
IC = NI // N_CORES   # input capsules per core = 144
NJ = IC // 4         # i-groups of 4 per core = 36
OD = NO * D          # 1024

F32 = mybir.dt.float32
ADD = mybir.AluOpType.add
MULT = mybir.AluOpType.mult
AXX = mybir.AxisListType.X
EXP = mybir.ActivationFunctionType.Exp

# s-accumulation strategy: PSUM_SACC=True accumulates c*x_hat in PSUM via
# identity-stationary PE matmuls (NGRP must be 3 to leave 2 banks free);
# False accumulates on DVE with tensor_add (NGRP=4, all 8 banks for x_hat).
PSUM_SACC = False
NGRP = 4
# Timing-ablation only: replace the cross-core AllReduce with a plain DMA
# (results become wrong; used to measure the collective's cost).
SKIP_COLLECTIVE = False


def _kernel_body(nc, tc, xs, ws, id_in, vout, repeats=1):
    with tc.tile_pool(name="main", bufs=1) as main, \
         tc.tile_pool(name="psum", bufs=1, space="PSUM") as psum, \
         tc.tile_pool(name="dram", bufs=1, space="DRAM") as dram:

        W_t = main.tile([128, NJ, OD], F32)
        x_t = main.tile([128, NJ, 128], F32)

        if PSUM_SACC:
            ident = main.tile([128, 128], F32)
            nc.sync.dma_start(ident[:], id_in[:])
            cx = main.tile([B, OD], F32)     # weighted x_hat staging for PE
            s_acc = None
        else:
            ident = cx = None
            s_acc = main.tile([B, OD], F32)
        v_cur = main.tile([B, OD], F32)
        s_full = main.tile([B, OD], F32)
        tmp2 = main.tile([B, 2 * OD], F32)   # scratch for 2-capsule slabs
        a4 = main.tile([B, NGRP * NO], F32)  # logits, NGRP capsules x 32 o
        a_old = main.tile([B, NGRP * NO], F32)
        e4 = main.tile([B, NGRP * NO], F32)
        c4 = main.tile([B, NGRP * NO], F32)
        z4 = main.tile([B, NGRP], F32)
        rz4 = main.tile([B, NGRP], F32)
        ones128 = main.tile([128, 1], F32)
        ones1 = main.tile([1, 128], F32)
        nrm1 = main.tile([1, 1], F32)
        s_sq = main.tile([B, 1], F32)
        g_sc = main.tile([B, 1], F32)
        t1 = main.tile([B, 1], F32)
        t2 = main.tile([B, 1], F32)
        t3 = main.tile([B, 1], F32)
        nc.vector.memset(ones128[:], 1.0)
        nc.vector.memset(ones1[:], 1.0)

        ar_in = dram.tile([B, OD], F32)
        ar_out = dram.tile([B, OD], F32)
        a_dram = dram.tile([B, IC // NGRP, NGRP * NO], F32)

        pg = psum.tile([B, NGRP * OD], F32)  # x_hat tiles
        s_ps = psum.tile([B, OD], F32) if PSUM_SACC else None

        def allreduce_s(src):
            nc.sync.dma_start(ar_in[:], src)
            if SKIP_COLLECTIVE:
                nc.sync.dma_start(ar_out[:], ar_in[:])
            else:
                nc.gpsimd.collective_compute(
                    "AllReduce", ADD,
                    replica_groups=[list(range(N_CORES))],
                    ins=[ar_in.opt()], outs=[ar_out.opt()],
                )
            nc.sync.dma_start(s_full[:], ar_out[:])

        def squash(accumulate=False):
            # g = sqrt(S)/(1+S), S = global sum of squares of s_full.
            # accumulate=False: v_cur = g*s_full.
            # accumulate=True:  v_cur += g*s_full (routing logits are linear
            # in v, so pass 2 can use v0+v1 as its agreement multiplier).
            nc.vector.tensor_mul(tmp2[:, 0:OD], s_full[:], s_full[:])
            # reduce over partitions via PE (ones stationary), then over free
            for h in range(2):
                nc.tensor.matmul(pg[0:1, 512 * h:512 * (h + 1)], ones128[:],
                                 tmp2[:, 512 * h:512 * (h + 1)],
                                 start=True, stop=True)
            nc.vector.tensor_reduce(out=nrm1[:], in_=pg[0:1, 0:OD],
                                    axis=AXX, op=ADD)
            # broadcast the scalar back to all 128 partitions via PE
            nc.tensor.matmul(pg[:, 0:1], ones1[:], nrm1[:],
                             start=True, stop=True)
            nc.vector.tensor_copy(s_sq[:], pg[:, 0:1])
            nc.scalar.sqrt(t1[:], s_sq[:])
            nc.vector.tensor_scalar_add(t2[:], s_sq[:], 1.0)
            nc.vector.reciprocal(t3[:], t2[:])
            nc.vector.tensor_mul(g_sc[:], t1[:], t3[:])
            if accumulate:
                nc.vector.tensor_scalar_mul(s_full[:], s_full[:], g_sc[:])
                nc.vector.tensor_add(v_cur[:], v_cur[:], s_full[:])
            else:
                nc.vector.tensor_scalar_mul(v_cur[:], s_full[:], g_sc[:])

        # ---- repeats > 1 is a timing aid: the whole computation re-runs
        # serially (same tiles, deps chain), so (T(R)-T(1))/(R-1) isolates
        # one full iteration including the W/x loads.
        for _rep in range(repeats):
            _run_once(nc, pg, s_ps, W_t, x_t, xs, ws, ident,
                      allreduce_s, squash, s_full, s_acc, v_cur,
                      tmp2, cx, a4, a_old, e4, c4, z4, rz4, a_dram)

        nc.sync.dma_start(vout[:], v_cur[:])


def _run_once(nc, pg, s_ps, W_t, x_t, xs, ws, ident, allreduce_s, squash,
              s_full, s_acc, v_cur, tmp2, cx, a4, a_old, e4, c4, z4, rz4,
              a_dram):
        nc.sync.dma_start(W_t[:], ws[:])
        nc.sync.dma_start(x_t[:], xs[:])

        # ---- pass 0: s0 = (1/32) sum_i x_hat[b,i,:,:], direct K=128 matmuls
        for h in range(2):
            for j in range(NJ):
                nc.tensor.matmul(
                    pg[:, 512 * h:512 * (h + 1)],
                    x_t[:, j, :], W_t[:, j, 512 * h:512 * (h + 1)],
                    start=(j == 0), stop=(j == NJ - 1))
        nc.vector.tensor_scalar_mul(tmp2[:, 0:OD], pg[:, 0:OD], 1.0 / NO)
        allreduce_s(tmp2[:, 0:OD])
        squash()

        # ---- passes 1, 2: groups of NGRP capsules; i -> (j=i//4, g=i%4)
        NG = IC // NGRP
        for r in (1, 2):
            if not PSUM_SACC:
                nc.vector.memset(s_acc[:], 0.0)
            for t in range(NG):
                # regenerate x_hat for NGRP capsules into PSUM (row-tiled)
                for slot in range(NGRP):
                    i = NGRP * t + slot
                    j, g = i // 4, i % 4
                    for h in range(2):
                        lo = slot * OD + 512 * h
                        nc.tensor.matmul(
                            pg[:, lo:lo + 512],
                            x_t[32 * g:32 * (g + 1), j, :],
                            W_t[32 * g:32 * (g + 1), j, 512 * h:512 * (h + 1)],
                            start=True, stop=True, tile_position=(32 * g, 0))
                # agreement logits: a4[b, (slot,o)] = sum_d x_hat * v_cur
                for half in range(NGRP // 2):
                    ph = pg[:, half * 2 * OD:(half + 1) * 2 * OD]
                    nc.vector.tensor_tensor(
                        out=tmp2[:].rearrange("b (i f) -> b i f", i=2),
                        in0=ph.rearrange("b (i f) -> b i f", i=2),
                        in1=v_cur[:].unsqueeze(1).broadcast_to([B, 2, OD]),
                        op=MULT)
                    nc.vector.tensor_reduce(
                        out=a4[:, half * 64:(half + 1) * 64]
                            .rearrange("b (i o) -> b i o", i=2),
                        in_=tmp2[:].rearrange("b (i o d) -> b i o d",
                                              i=2, o=NO),
                        axis=AXX, op=ADD)
                # c4 = softmax over o (logits are tiny; skip max-subtraction)
                nc.scalar.activation(e4[:], a4[:], EXP)
                nc.vector.tensor_reduce(
                    out=z4[:], in_=e4[:].rearrange("b (i o) -> b i o", i=NGRP),
                    axis=AXX, op=ADD)
                nc.vector.reciprocal(rz4[:], z4[:])
                nc.vector.tensor_tensor(
                    out=c4[:].rearrange("b (i o) -> b i o", i=NGRP),
                    in0=e4[:].rearrange("b (i o) -> b i o", i=NGRP),
                    in1=rz4[:].unsqueeze(2).broadcast_to([B, NGRP, NO]),
                    op=MULT)
                # s += c*x_hat: one double-width multiply per capsule PAIR
                # (fewer DVE ops -> fewer pipeline DRAINs), adds on GpSimd
                if PSUM_SACC:
                    for slot in range(NGRP):
                        nc.vector.tensor_tensor(
                            out=cx[:].rearrange("b (o d) -> b o d", o=NO),
                            in0=pg[:, slot * OD:(slot + 1) * OD]
                                .rearrange("b (o d) -> b o d", o=NO),
                            in1=c4[:, slot * NO:(slot + 1) * NO]
                                .unsqueeze(2).broadcast_to([B, NO, D]),
                            op=MULT)
                        first = (t == 0 and slot == 0)
                        last = (t == NG - 1 and slot == NGRP - 1)
                        for h in range(2):
                            nc.tensor.matmul(
                                s_ps[:, 512 * h:512 * (h + 1)], ident[:],
                                cx[:, 512 * h:512 * (h + 1)],
                                start=first, stop=last)
                else:
                    for half in range(NGRP // 2):
                        nc.vector.tensor_tensor(
                            out=tmp2[:].rearrange("b (i o d) -> b i o d",
                                                  i=2, o=NO),
                            in0=pg[:, half * 2 * OD:(half + 1) * 2 * OD]
                                .rearrange("b (i o d) -> b i o d", i=2, o=NO),
                            in1=c4[:, half * 64:(half + 1) * 64]
                                .rearrange("b (i o) -> b i o", i=2)
                                .unsqueeze(3).broadcast_to([B, 2, NO, D]),
                            op=MULT)
                        nc.gpsimd.tensor_add(s_acc[:], s_acc[:],
                                             tmp2[:, 0:OD])
                        nc.gpsimd.tensor_add(s_acc[:], s_acc[:],
                                             tmp2[:, OD:2 * OD])
            if PSUM_SACC:
                nc.vector.tensor_copy(tmp2[:, 0:OD], s_ps[:])
                allreduce_s(tmp2[:, 0:OD])
            else:
                allreduce_s(s_acc[:])
            # pass 1: v_cur <- v0 + v1 (joint agreement multiplier for
            # pass 2); pass 2: v_cur <- v2 (the output)
            squash(accumulate=(r == 1))


_NC_CACHE = {}


def _build(repeats=1):
    if repeats in _NC_CACHE:
        return _NC_CACHE[repeats]
    nc = bacc.Bacc("TRN2", target_bir_lowering=False, debug=False,
                   num_devices=N_CORES)
    xs = nc.dram_tensor("xs", [128, NJ, 128], F32, kind="ExternalInput").ap()
    ws = nc.dram_tensor("ws", [128, NJ, OD], F32, kind="ExternalInput").ap()
    id_in = nc.dram_tensor("ident", [128, 128], F32, kind="ExternalInput").ap()
    vout = nc.dram_tensor("v", [B, OD], F32, kind="ExternalOutput").ap()
    with tile.TileContext(nc) as tc:
        _kernel_body(nc, tc, xs, ws, id_in, vout, repeats=repeats)
    nc.compile()
    _NC_CACHE[repeats] = nc
    return nc


def _shard_inputs(x, W):
    in_maps = []
    for c in range(N_CORES):
        i0 = c * IC
        wc = W[i0:i0 + IC]                          # (144, 32, 32, 32) iodk
        ws = np.ascontiguousarray(
            wc.reshape(NJ, 4, NO, D, K).transpose(1, 4, 0, 2, 3)
              .reshape(128, NJ, OD)).astype(np.float32, copy=False)
        xc = x[:, i0:i0 + IC, :]                    # (128, 144, 32) bik
        xt = np.ascontiguousarray(
            xc.reshape(B, NJ, 4, K).transpose(2, 3, 1, 0)
              .reshape(128, NJ, 128)).astype(np.float32, copy=False)
        in_maps.append({"xs": xt, "ws": ws,
                        "ident": np.eye(128, dtype=np.float32)})
    return in_maps


def kernel(x, W, _trace=False):
    x = np.asarray(x, dtype=np.float32)
    W = np.asarray(W, dtype=np.float32)
    nc = _build()
    in_maps = _shard_inputs(x, W)
    res = bass_utils.run_bass_kernel_spmd(
        nc, in_maps, core_ids=list(range(N_CORES)), trace=_trace)
    out = res.results[0]["v"].reshape(B, NO, D).astype(np.float32, copy=False)
    if _trace:
        kernel.last_exec_time_ns = res.exec_time_ns
        kernel.last_results = res
    return out



# revision 11
# speedup vs baseline: 2.1093x; 2.1093x over previous
"""Trainium2 Bass kernel for CapsNet dynamic routing (nn_CapsRoutingLayer).

Reference computation:
    x_hat[b,i,o,d] = sum_k W[i,o,d,k] * x[b,i,k]
    b_logits = 0
    for it in 0..2:
        c = softmax_o(b_logits); s[b,o,d] = sum_i c[b,i,o] x_hat[b,i,o,d]
        v = squash(s)   # global Frobenius norm over the whole s tensor
        if it < 2: b_logits += sum_d x_hat[b,i,o,d] v[b,o,d]
    return v  # (128, 32, 32)

Sharding: input capsules i (1152) split across 8 cores (144 each); the tiny
[128,1024] per-iteration s is AllReduced. Per-core strategy:

  * All matmuls in bf16 (PE runs fp32 at 1/4 rate). W shard (9.4MB bf16)
    stays resident in SBUF, loaded in 4 chunks so pass-0 starts early.
  * Pass 0 (uniform c): s0 = (1/32) sum_i x_hat via K=128 matmuls.
  * Passes 1,2 regenerate x_hat per supergroup of 8 capsules into PSUM
    (2-capsule tiles, double-buffered), the Scalar engine copies it to SBUF
    as bf16, and all elementwise work runs on DVE in its 2x/4x 16-bit modes.
    Per-capsule s contributions are pair-summed in a small bf16 add tree on
    DVE; GpSimd (0.42 add efficiency) only accumulates one fp32 add per 16
    capsules.
  * squash's global norm is PSUM-free: tensor_tensor_reduce + GpSimd
    partition reduce/broadcast. Routing logits are linear in v, so pass 2
    uses m2 = v0 + v1 as its agreement multiplier (b2 = <x_hat, v0+v1>).

Software pipeline (emission order == per-engine execution order): supergroup
S's PE regen + ACT copies are emitted one round ahead of its softmax and s
accumulation, so no engine stalls on another's tail.
"""

import numpy as np
import ml_dtypes

from concourse import bacc, bass_isa, bass_utils, mybir, tile

N_CORES = 8
B = 128          # batch
NI = 1152        # input capsules
K = 32           # dim_input
NO = 32          # output capsules
D = 32           # dim_output
IC = NI // N_CORES   # input capsules per core = 144
NJ = IC // 4         # i-groups of 4 per core = 36
OD = NO * D          # 1024
SGC = 8              # capsules per supergroup
NSG = IC // SGC      # supergroups per pass = 18
NWC = 4              # W is DMA'd in NWC chunks of NJ//NWC j-groups
JC = NJ // NWC       # 9

F32 = mybir.dt.float32
BF16 = mybir.dt.bfloat16
ADD = mybir.AluOpType.add
MULT = mybir.AluOpType.mult
AXX = mybir.AxisListType.X
AXC = mybir.AxisListType.C
EXP = mybir.ActivationFunctionType.Exp

# Timing-ablation only: replace the cross-core AllReduce with a plain DMA
# (results become wrong; used to measure the collective's cost).
SKIP_COLLECTIVE = False
# Debug toggles for HW bisection.
USE_TTR = False     # tensor_tensor_reduce in squash
USE_STT = False     # scalar_tensor_tensor for pass-2 multiplier


def _kernel_body(nc, tc, xs, ws, vout, repeats=1):
    with tc.tile_pool(name="persist", bufs=1) as per, \
         tc.tile_pool(name="xhp", bufs=2) as xhp, \
         tc.tile_pool(name="smallp", bufs=2) as smallp, \
         tc.tile_pool(name="pp", bufs=3) as pp, \
         tc.tile_pool(name="pgp", bufs=2, space="PSUM") as pgp, \
         tc.tile_pool(name="dram", bufs=1, space="DRAM") as dram:

        W_t = [per.tile([128, JC, OD], BF16, name=f"W_t{c}") for c in range(NWC)]
        x_t = per.tile([128, NJ, 128], BF16)
        for c in range(NWC):
            nc.sync.dma_start(W_t[c][:], ws[:, JC * c:JC * (c + 1), :])
        nc.sync.dma_start(x_t[:], xs[:])

        xv = per.tile([B, SGC * OD], BF16)     # agreement products (DVE-only)
        sx = per.tile([B, SGC * OD], BF16)     # c-weighted x_hat (DVE-only)
        s_acc = per.tile([B, OD], F32)         # local s accumulator
        s_loc = per.tile([B, OD], F32)         # pass-0 staging
        s_full = per.tile([B, OD], F32)        # post-AllReduce s
        v0 = per.tile([B, OD], F32)            # squash(s0)
        v_out = per.tile([B, OD], F32)         # final output
        m_bf = per.tile([B, OD], BF16)         # agreement multiplier (bf16)
        sq = per.tile([B, OD], F32)            # squash scratch
        col = per.tile([B, 1], F32)
        Sn1 = per.tile([1, 1], F32)
        Sb = per.tile([128, 1], F32)
        t1 = per.tile([128, 1], F32)
        t2 = per.tile([128, 1], F32)
        t3 = per.tile([128, 1], F32)
        gb = per.tile([128, 1], F32)
        ones128 = per.tile([128, 1], F32)
        ones1 = per.tile([1, 128], F32)
        nc.vector.memset(ones128[:], 1.0)
        nc.vector.memset(ones1[:], 1.0)

        ar_in = dram.tile([B, OD], F32)
        ar_out = dram.tile([B, OD], F32)

        def allreduce(src):
            nc.sync.dma_start(ar_in[:], src[:])
            if SKIP_COLLECTIVE:
                nc.sync.dma_start(ar_out[:], ar_in[:])
            else:
                nc.gpsimd.collective_compute(
                    "AllReduce", ADD,
                    replica_groups=[list(range(N_CORES))],
                    ins=[ar_in.opt()], outs=[ar_out.opt()],
                )
            nc.sync.dma_start(s_full[:], ar_out[:])

        def squash_mult(pass_idx):
            # g = sqrt(S)/(1+S) with S = global sum of squares of s_full,
            # then update the next pass's agreement multiplier / output.
            if USE_TTR:
                nc.vector.tensor_tensor_reduce(
                    out=sq[:], in0=s_full[:], in1=s_full[:], scale=1.0,
                    scalar=0.0, op0=MULT, op1=ADD, accum_out=col[:])
            else:
                nc.vector.tensor_mul(sq[:], s_full[:], s_full[:])
                nc.vector.tensor_reduce(out=col[:], in_=sq[:], axis=AXX,
                                        op=ADD)
            # partition reduce + broadcast via PE with ones (1-row matmuls)
            ps = pgp.tile([B, 2 * OD], F32, name="ps", tag="pg")
            nc.tensor.matmul(ps[0:1, 0:1], ones128[:], col[:],
                             start=True, stop=True)
            nc.vector.tensor_copy(Sn1[:], ps[0:1, 0:1])
            nc.tensor.matmul(ps[:, 512:513], ones1[:], Sn1[:],
                             start=True, stop=True)
            nc.vector.tensor_copy(Sb[:], ps[:, 512:513])
            nc.scalar.sqrt(t1[:], Sb[:])
            nc.vector.tensor_scalar_add(t2[:], Sb[:], 1.0)
            nc.vector.reciprocal(t3[:], t2[:])
            nc.vector.tensor_mul(gb[:], t1[:], t3[:])
            if pass_idx == 0:
                nc.vector.tensor_scalar_mul(v0[:], s_full[:], gb[:])
                nc.vector.tensor_copy(m_bf[:], v0[:])          # m1 = v0
            elif pass_idx == 1:
                if USE_STT:
                    nc.vector.scalar_tensor_tensor(             # m2 = g1*s1+v0
                        out=m_bf[:], in0=s_full[:], scalar=gb[:], in1=v0[:],
                        op0=MULT, op1=ADD)
                else:
                    nc.vector.tensor_scalar_mul(sq[:], s_full[:], gb[:])
                    nc.vector.tensor_add(m_bf[:], sq[:], v0[:])
            else:
                nc.vector.tensor_scalar_mul(v_out[:], s_full[:], gb[:])

        def regen_supergroup(S):
            # PE: x_hat for capsules 8S..8S+7 -> PSUM; ACT: cast-copy to SBUF
            xh = xhp.tile([B, SGC * OD], BF16, name="xh", tag="xh")
            for g4 in range(SGC // 2):
                pg = pgp.tile([B, 2 * OD], F32, name="pg", tag="pg")
                for slot in range(2):
                    i = SGC * S + 2 * g4 + slot
                    j, gg = divmod(i, 4)
                    wt = W_t[j // JC]
                    jj = j % JC
                    for h in range(2):
                        lo = slot * OD + 512 * h
                        nc.tensor.matmul(
                            pg[:, lo:lo + 512],
                            x_t[32 * gg:32 * (gg + 1), j, :],
                            wt[32 * gg:32 * (gg + 1), jj, 512 * h:512 * (h + 1)],
                            start=True, stop=True, tile_position=(32 * gg, 0))
                nc.scalar.copy(xh[:, 2 * OD * g4:2 * OD * (g4 + 1)], pg[:])
            return xh

        def run_pass(r):
            state = {}
            for S in range(NSG + 1):
                if S < NSG:
                    xh = regen_supergroup(S)
                    # agreement multiply: xv = xh * m (broadcast over capsule)
                    nc.vector.tensor_tensor(
                        out=xv[:].rearrange("b (i f) -> b i f", i=SGC),
                        in0=xh[:].rearrange("b (i f) -> b i f", i=SGC),
                        in1=m_bf[:].unsqueeze(1).broadcast_to([B, SGC, OD]),
                        op=MULT)
                if S >= 1:
                    st = state[S - 1]
                    nc.vector.tensor_reduce(
                        out=st["z"][:],
                        in_=st["e"][:].rearrange("b (i o) -> b i o", i=SGC),
                        axis=AXX, op=ADD)
                    nc.vector.reciprocal(st["rz"][:], st["z"][:])
                if S < NSG:
                    a16 = smallp.tile([B, SGC * NO], BF16, name="a16", tag="a")
                    nc.vector.tensor_reduce(
                        out=a16[:].rearrange("b (i o) -> b i o", i=SGC),
                        in_=xv[:].rearrange("b (i o d) -> b i o d", i=SGC, o=NO),
                        axis=AXX, op=ADD)
                    e16 = smallp.tile([B, SGC * NO], BF16, name="e16", tag="e")
                    nc.scalar.activation(e16[:], a16[:], EXP)
                    z16 = smallp.tile([B, SGC], F32, name="z16", tag="z")
                    rz16 = smallp.tile([B, SGC], F32, name="rz16", tag="rz")
                    state[S] = dict(e=e16, z=z16, rz=rz16, xh=xh)
                if S >= 1:
                    st = state[S - 1]
                    c16 = smallp.tile([B, SGC * NO], BF16, name="c16", tag="c")
                    nc.vector.tensor_tensor(
                        out=c16[:].rearrange("b (i o) -> b i o", i=SGC),
                        in0=st["e"][:].rearrange("b (i o) -> b i o", i=SGC),
                        in1=st["rz"][:].unsqueeze(2).broadcast_to([B, SGC, NO]),
                        op=MULT)
                    # s contribution: sx = xh * c (broadcast over d)
                    nc.vector.tensor_tensor(
                        out=sx[:].rearrange("b (i o d) -> b i o d", i=SGC, o=NO),
                        in0=st["xh"][:].rearrange("b (i o d) -> b i o d",
                                                  i=SGC, o=NO),
                        in1=c16[:].rearrange("b (i o) -> b i o", i=SGC)
                            .unsqueeze(3).broadcast_to([B, SGC, NO, D]),
                        op=MULT)
                    # bf16 add tree: 8 capsule blocks -> 1
                    nc.vector.tensor_add(sx[:, 0:2 * OD], sx[:, 0:2 * OD],
                                         sx[:, 2 * OD:4 * OD])
                    nc.vector.tensor_add(sx[:, 0:2 * OD], sx[:, 0:2 * OD],
                                         sx[:, 4 * OD:6 * OD])
                    nc.vector.tensor_add(sx[:, 0:2 * OD], sx[:, 0:2 * OD],
                                         sx[:, 6 * OD:8 * OD])
                    p16 = pp.tile([B, OD], BF16, name="p16", tag="p")
                    nc.vector.tensor_add(p16[:], sx[:, 0:OD], sx[:, OD:2 * OD])
                    st["p"] = p16
                    if (S - 1) % 2 == 1:
                        if S - 1 == 1:
                            # first pair: s_acc = p0 + p1 (InstTensorTensor,
                            # the only gpsimd op in the standard ucode lib)
                            nc.gpsimd.tensor_add(s_acc[:],
                                                 state[S - 2]["p"][:], p16[:])
                        else:
                            nc.vector.tensor_add(p16[:], p16[:],
                                                 state[S - 2]["p"][:])
                            nc.gpsimd.tensor_add(s_acc[:], s_acc[:], p16[:])
            allreduce(s_acc)
            squash_mult(r)

        with nc.allow_low_precision("bf16 routing; tolerance is 2e-2"):
            for _rep in range(repeats):
                # ---- pass 0: s0 = (1/32) sum_i x_hat, K=128 matmuls
                pg0 = pgp.tile([B, 2 * OD], F32, name="pg0", tag="pg")
                for j in range(NJ):
                    wt = W_t[j // JC]
                    jj = j % JC
                    for h in range(2):
                        nc.tensor.matmul(
                            pg0[:, 512 * h:512 * (h + 1)],
                            x_t[:, j, :], wt[:, jj, 512 * h:512 * (h + 1)],
                            start=(j == 0), stop=(j == NJ - 1))
                nc.vector.tensor_scalar_mul(s_loc[:], pg0[:, 0:OD], 1.0 / NO)
                allreduce(s_loc)
                squash_mult(0)
                run_pass(1)
                run_pass(2)

        nc.sync.dma_start(vout[:], v_out[:])


_NC_CACHE = {}


def _build(repeats=1):
    if repeats in _NC_CACHE:
        return _NC_CACHE[repeats]
    nc = bacc.Bacc("TRN2", target_bir_lowering=False, debug=False,
                   num_devices=N_CORES)
    xs = nc.dram_tensor("xs", [128, NJ, 128], BF16, kind="ExternalInput").ap()
    ws = nc.dram_tensor("ws", [128, NJ, OD], BF16, kind="ExternalInput").ap()
    vout = nc.dram_tensor("v", [B, OD], F32, kind="ExternalOutput").ap()
    with tile.TileContext(nc) as tc:
        _kernel_body(nc, tc, xs, ws, vout, repeats=repeats)
    nc.compile()
    _NC_CACHE[repeats] = nc
    return nc


def _shard_inputs(x, W):
    BF = ml_dtypes.bfloat16
    in_maps = []
    for c in range(N_CORES):
        i0 = c * IC
        wc = W[i0:i0 + IC]                          # (144, 32, 32, 32) iodk
        wsn = np.ascontiguousarray(
            wc.reshape(NJ, 4, NO, D, K).transpose(1, 4, 0, 2, 3)
              .reshape(128, NJ, OD)).astype(BF)
        xc = x[:, i0:i0 + IC, :]                    # (128, 144, 32) bik
        xt = np.ascontiguousarray(
            xc.reshape(B, NJ, 4, K).transpose(2, 3, 1, 0)
              .reshape(128, NJ, 128)).astype(BF)
        in_maps.append({"xs": xt, "ws": wsn})
    return in_maps


def kernel(x, W, _trace=False):
    x = np.asarray(x, dtype=np.float32)
    W = np.asarray(W, dtype=np.float32)
    nc = _build()
    in_maps = _shard_inputs(x, W)
    res = bass_utils.run_bass_kernel_spmd(
        nc, in_maps, core_ids=list(range(N_CORES)), trace=_trace)
    out = res.results[0]["v"].reshape(B, NO, D).astype(np.float32, copy=False)
    if _trace:
        kernel.last_exec_time_ns = res.exec_time_ns
        kernel.last_results = res
    return out


# revision 24
# speedup vs baseline: 3.6030x; 1.7082x over previous
"""Trainium2 Bass kernel for CapsNet dynamic routing (nn_CapsRoutingLayer).

Reference computation:
    x_hat[b,i,o,d] = sum_k W[i,o,d,k] * x[b,i,k]
    b_logits = 0
    for it in 0..2:
        c = softmax_o(b_logits); s[b,o,d] = sum_i c[b,i,o] x_hat[b,i,o,d]
        v = squash(s)   # global Frobenius norm over the whole s tensor
        if it < 2: b_logits += sum_d x_hat[b,i,o,d] v[b,o,d]
    return v  # (128, 32, 32)

Sharding: input capsules i (1152) split across 8 cores (144 each); the tiny
[128,1024] per-iteration s is AllReduced. Per-core strategy:

  * All matmuls in bf16 (PE runs fp32 at 1/4 rate). W shard (9.4MB bf16)
    stays resident in SBUF, loaded in 4 chunks so pass-0 starts early.
  * Pass 0 (uniform c): s0 = (1/32) sum_i x_hat via K=128 matmuls.
  * Passes 1,2 regenerate x_hat per supergroup of 8 capsules into PSUM
    (2-capsule tiles, double-buffered), the Scalar engine copies it to SBUF
    as bf16, and all elementwise work runs on DVE in its 2x/4x 16-bit modes.
    Per-capsule s contributions are pair-summed in a small bf16 add tree on
    DVE; GpSimd (0.42 add efficiency) only accumulates one fp32 add per 16
    capsules.
  * squash's global norm is PSUM-free: tensor_tensor_reduce + GpSimd
    partition reduce/broadcast. Routing logits are linear in v, so pass 2
    uses m2 = v0 + v1 as its agreement multiplier (b2 = <x_hat, v0+v1>).

Software pipeline (emission order == per-engine execution order): supergroup
S's PE regen + ACT copies are emitted one round ahead of its softmax and s
accumulation, so no engine stalls on another's tail.
"""

import numpy as np
import ml_dtypes

from concourse import bacc, bass_isa, bass_utils, mybir, tile

N_CORES = 8
B = 128          # batch
NI = 1152        # input capsules
K = 32           # dim_input
NO = 32          # output capsules
D = 32           # dim_output
IC = NI // N_CORES   # input capsules per core = 144
NJ = IC // 4         # i-groups of 4 per core = 36
OD = NO * D          # 1024
SGC = 8              # capsules per supergroup
NSG = IC // SGC      # supergroups per pass = 18
NWC = 4              # W is DMA'd in NWC chunks of NJ//NWC j-groups
JC = NJ // NWC       # 9

F32 = mybir.dt.float32
BF16 = mybir.dt.bfloat16
ADD = mybir.AluOpType.add
MULT = mybir.AluOpType.mult
AXX = mybir.AxisListType.X
AXC = mybir.AxisListType.C
EXP = mybir.ActivationFunctionType.Exp

# Timing-ablation only: replace the cross-core AllReduce with a plain DMA
# (results become wrong; used to measure the collective's cost).
SKIP_COLLECTIVE = False
# Debug toggles for HW bisection.
USE_TTR = False     # tensor_tensor_reduce in squash
USE_STT = True


def _kernel_body(nc, tc, xs, ws, vout, repeats=1):
    with tc.tile_pool(name="persist", bufs=1) as per, \
         tc.tile_pool(name="xhp", bufs=3) as xhp, \
         tc.tile_pool(name="smallp", bufs=2) as smallp, \
         tc.tile_pool(name="pgp", bufs=2, space="PSUM") as pgp, \
         tc.tile_pool(name="dram", bufs=1, space="DRAM") as dram:

        W_t = [per.tile([128, JC, OD], BF16, name=f"W_t{c}") for c in range(NWC)]
        x_t = per.tile([128, NJ, 128], BF16)
        nc.sync.dma_start(x_t[:], xs[:])       # small; every matmul needs it
        for c in range(NWC):
            nc.sync.dma_start(W_t[c][:], ws[:, JC * c:JC * (c + 1), :])

        xv = per.tile([B, SGC * OD], BF16)     # agreement products (DVE-only)
        sx = per.tile([B, SGC * OD], BF16)     # c-weighted x_hat (DVE-only)
        s_acc = per.tile([B, OD], F32)         # local s accumulator
        s_loc = per.tile([B, OD], F32)         # pass-0 staging
        s_full = per.tile([B, OD], F32)        # post-AllReduce s
        v0 = per.tile([B, OD], F32)            # squash(s0)
        v_out = per.tile([B, OD], F32)         # final output
        m_bf = per.tile([B, OD], BF16)         # agreement multiplier (bf16)
        sq = per.tile([B, OD], F32)            # squash scratch
        col = per.tile([B, 1], F32)
        Sn1 = per.tile([1, 1], F32)
        Sb = per.tile([128, 1], F32)
        t1 = per.tile([128, 1], F32)
        t2 = per.tile([128, 1], F32)
        t3 = per.tile([128, 1], F32)
        gb = per.tile([128, 1], F32)
        ones128 = per.tile([128, 1], F32)
        ones1 = per.tile([1, 128], F32)
        nc.vector.memset(ones128[:], 1.0)
        nc.vector.memset(ones1[:], 1.0)

        ar_in = dram.tile([B, OD], F32)
        ar_out = dram.tile([B, OD], F32)

        def allreduce(src):
            nc.sync.dma_start(ar_in[:], src[:])
            if SKIP_COLLECTIVE:
                nc.sync.dma_start(ar_out[:], ar_in[:])
            else:
                nc.gpsimd.collective_compute(
                    "AllReduce", ADD,
                    replica_groups=[list(range(N_CORES))],
                    ins=[ar_in.opt()], outs=[ar_out.opt()],
                )
            nc.sync.dma_start(s_full[:], ar_out[:])

        def squash_mult(pass_idx):
            # g = sqrt(S)/(1+S) with S = global sum of squares of s_full,
            # then update the next pass's agreement multiplier / output.
            if USE_TTR:
                nc.vector.tensor_tensor_reduce(
                    out=sq[:], in0=s_full[:], in1=s_full[:], scale=1.0,
                    scalar=0.0, op0=MULT, op1=ADD, accum_out=col[:])
            else:
                nc.vector.tensor_mul(sq[:], s_full[:], s_full[:])
                nc.vector.tensor_reduce(out=col[:], in_=sq[:], axis=AXX,
                                        op=ADD)
            # partition reduce + broadcast via PE with ones (1-row matmuls)
            ps = pgp.tile([B, 2 * OD], F32, name="ps", tag="pg")
            nc.tensor.matmul(ps[0:1, 0:1], ones128[:], col[:],
                             start=True, stop=True)
            nc.vector.tensor_copy(Sn1[:], ps[0:1, 0:1])
            nc.tensor.matmul(ps[:, 512:513], ones1[:], Sn1[:],
                             start=True, stop=True)
            nc.vector.tensor_copy(Sb[:], ps[:, 512:513])
            nc.scalar.sqrt(t1[:], Sb[:])
            nc.vector.tensor_scalar_add(t2[:], Sb[:], 1.0)
            nc.vector.reciprocal(t3[:], t2[:])
            nc.vector.tensor_mul(gb[:], t1[:], t3[:])
            if pass_idx == 0:
                nc.vector.tensor_scalar_mul(v0[:], s_full[:], gb[:])
                nc.vector.tensor_copy(m_bf[:], v0[:])          # m1 = v0
            elif pass_idx == 1:
                if USE_STT:
                    nc.vector.scalar_tensor_tensor(             # m2 = g1*s1+v0
                        out=m_bf[:], in0=s_full[:], scalar=gb[:], in1=v0[:],
                        op0=MULT, op1=ADD)
                else:
                    nc.vector.tensor_scalar_mul(sq[:], s_full[:], gb[:])
                    nc.vector.tensor_add(m_bf[:], sq[:], v0[:])
            else:
                nc.vector.tensor_scalar_mul(v_out[:], s_full[:], gb[:])

        def regen_supergroup(S):
            # PE: x_hat for capsules 8S..8S+7 -> PSUM; ACT: cast-copy to SBUF
            xh = xhp.tile([B, SGC * OD], BF16, name="xh", tag="xh")
            for g4 in range(SGC // 2):
                pg = pgp.tile([B, 2 * OD], F32, name="pg", tag="pg")
                for slot in range(2):
                    i = SGC * S + 2 * g4 + slot
                    j, gg = divmod(i, 4)
                    wt = W_t[j // JC]
                    jj = j % JC
                    for h in range(2):
                        lo = slot * OD + 512 * h
                        nc.tensor.matmul(
                            pg[:, lo:lo + 512],
                            x_t[32 * gg:32 * (gg + 1), j, :],
                            wt[32 * gg:32 * (gg + 1), jj, 512 * h:512 * (h + 1)],
                            start=True, stop=True, tile_position=(32 * gg, 0))
                nc.scalar.copy(xh[:, 2 * OD * g4:2 * OD * (g4 + 1)], pg[:])
            return xh

        def run_pass(r):
            state = {}
            for S in range(NSG + 1):
                if S < NSG:
                    xh = regen_supergroup(S)
                    # agreement multiply: xv = xh * m (broadcast over capsule)
                    nc.vector.tensor_tensor(
                        out=xv[:].rearrange("b (i f) -> b i f", i=SGC),
                        in0=xh[:].rearrange("b (i f) -> b i f", i=SGC),
                        in1=m_bf[:].unsqueeze(1).broadcast_to([B, SGC, OD]),
                        op=MULT)
                if S >= 1:
                    st = state[S - 1]
                    nc.vector.tensor_reduce(
                        out=st["z"][:],
                        in_=st["e"][:].rearrange("b (i o) -> b i o", i=SGC),
                        axis=AXX, op=ADD)
                    nc.vector.reciprocal(st["rz"][:], st["z"][:])
                if S < NSG:
                    # reduce over d (the MIDDLE axis in the [b,i,d,o] layout,
                    # so every fold keeps a packed o innermost and runs in the
                    # DVE 2x mode; TensorReduce has no fast mode at all) as a
                    # log2 fold tree of adds.
                    a16 = smallp.tile([B, SGC * NO], BF16, name="a16", tag="a")
                    v4 = xv[:].rearrange("b (i d o) -> b i d o", i=SGC, d=D)
                    w = D // 2
                    while w > 1:
                        nc.vector.tensor_add(v4[:, :, 0:w, :], v4[:, :, 0:w, :],
                                             v4[:, :, w:2 * w, :])
                        w //= 2
                    nc.vector.tensor_add(
                        a16[:].rearrange("b (i o) -> b i o", i=SGC)
                            .unsqueeze(2),
                        v4[:, :, 0:1, :], v4[:, :, 1:2, :])
                    e16 = smallp.tile([B, SGC * NO], BF16, name="e16", tag="e")
                    nc.scalar.activation(e16[:], a16[:], EXP)
                    z16 = smallp.tile([B, SGC], F32, name="z16", tag="z")
                    rz16 = smallp.tile([B, SGC], F32, name="rz16", tag="rz")
                    state[S] = dict(e=e16, z=z16, rz=rz16, xh=xh)
                if S >= 1:
                    st = state[S - 1]
                    c16 = smallp.tile([B, SGC * NO], BF16, name="c16", tag="c")
                    nc.vector.tensor_tensor(
                        out=c16[:].rearrange("b (i o) -> b i o", i=SGC),
                        in0=st["e"][:].rearrange("b (i o) -> b i o", i=SGC),
                        in1=st["rz"][:].unsqueeze(2).broadcast_to([B, SGC, NO]),
                        op=MULT)
                    # s contribution: sx = xh * c (broadcast over d; d is the
                    # middle axis so the innermost o stays packed -> DVE 2x)
                    nc.vector.tensor_tensor(
                        out=sx[:].rearrange("b (i d o) -> b i d o", i=SGC, d=D),
                        in0=st["xh"][:].rearrange("b (i d o) -> b i d o",
                                                  i=SGC, d=D),
                        in1=c16[:].rearrange("b (i o) -> b i o", i=SGC)
                            .unsqueeze(2).broadcast_to([B, SGC, D, NO]),
                        op=MULT)
                    # capsule-sum tree, split DVE (blocks 0-4) / GpSimd (5-7);
                    # GpSimd then owns the serial fp32 s_acc accumulation, so
                    # DVE never waits on the slower engine.
                    nc.vector.tensor_add(sx[:, 0:OD], sx[:, 0:OD],
                                         sx[:, OD:2 * OD])
                    nc.vector.tensor_add(sx[:, 2 * OD:3 * OD],
                                         sx[:, 2 * OD:3 * OD],
                                         sx[:, 3 * OD:4 * OD])
                    nc.vector.tensor_add(sx[:, 0:OD], sx[:, 0:OD],
                                         sx[:, 2 * OD:3 * OD])
                    nc.vector.tensor_add(sx[:, 0:OD], sx[:, 0:OD],
                                         sx[:, 4 * OD:5 * OD])
                    nc.gpsimd.tensor_add(sx[:, 5 * OD:6 * OD],
                                         sx[:, 5 * OD:6 * OD],
                                         sx[:, 6 * OD:7 * OD])
                    nc.gpsimd.tensor_add(sx[:, 5 * OD:6 * OD],
                                         sx[:, 5 * OD:6 * OD],
                                         sx[:, 7 * OD:8 * OD])
                    if S - 1 == 0:
                        nc.gpsimd.tensor_add(s_acc[:], sx[:, 0:OD],
                                             sx[:, 5 * OD:6 * OD])
                    else:
                        nc.gpsimd.tensor_add(s_acc[:], s_acc[:], sx[:, 0:OD])
                        nc.gpsimd.tensor_add(s_acc[:], s_acc[:],
                                             sx[:, 5 * OD:6 * OD])
            allreduce(s_acc)
            squash_mult(r)

        with nc.allow_low_precision("bf16 routing; tolerance is 2e-2"):
            for _rep in range(repeats):
                # ---- pass 0: s0 = (1/32) sum_i x_hat, K=128 matmuls
                pg0 = pgp.tile([B, 2 * OD], F32, name="pg0", tag="pg")
                for j in range(NJ):
                    wt = W_t[j // JC]
                    jj = j % JC
                    for h in range(2):
                        nc.tensor.matmul(
                            pg0[:, 512 * h:512 * (h + 1)],
                            x_t[:, j, :], wt[:, jj, 512 * h:512 * (h + 1)],
                            start=(j == 0), stop=(j == NJ - 1))
                nc.vector.tensor_scalar_mul(s_loc[:], pg0[:, 0:OD], 1.0 / NO)
                allreduce(s_loc)
                squash_mult(0)
                run_pass(1)
                run_pass(2)

        nc.sync.dma_start(vout[:], v_out[:])


_NC_CACHE = {}


def _build(repeats=1):
    if repeats in _NC_CACHE:
        return _NC_CACHE[repeats]
    nc = bacc.Bacc("TRN2", target_bir_lowering=False, debug=False,
                   num_devices=N_CORES)
    xs = nc.dram_tensor("xs", [128, NJ, 128], BF16, kind="ExternalInput").ap()
    ws = nc.dram_tensor("ws", [128, NJ, OD], BF16, kind="ExternalInput").ap()
    vout = nc.dram_tensor("v", [B, OD], F32, kind="ExternalOutput").ap()
    with tile.TileContext(nc) as tc:
        _kernel_body(nc, tc, xs, ws, vout, repeats=repeats)
    nc.compile()
    _NC_CACHE[repeats] = nc
    return nc


def _shard_inputs(x, W):
    BF = ml_dtypes.bfloat16
    in_maps = []
    for c in range(N_CORES):
        i0 = c * IC
        wc = W[i0:i0 + IC]                          # (144, 32, 32, 32) iodk
        # (d,o)-transposed columns: ws[(g,k), j, (d,o)] = W[i0+4j+g, o, d, k]
        wsn = np.ascontiguousarray(
            wc.reshape(NJ, 4, NO, D, K).transpose(1, 4, 0, 3, 2)
              .reshape(128, NJ, OD)).astype(BF)
        xc = x[:, i0:i0 + IC, :]                    # (128, 144, 32) bik
        xt = np.ascontiguousarray(
            xc.reshape(B, NJ, 4, K).transpose(2, 3, 1, 0)
              .reshape(128, NJ, 128)).astype(BF)
        in_maps.append({"xs": xt, "ws": wsn})
    return in_maps


def kernel(x, W, _trace=False):
    x = np.asarray(x, dtype=np.float32)
    W = np.asarray(W, dtype=np.float32)
    nc = _build()
    in_maps = _shard_inputs(x, W)
    res = bass_utils.run_bass_kernel_spmd(
        nc, in_maps, core_ids=list(range(N_CORES)), trace=_trace)
    # kernel works in (d,o)-transposed layout; untranspose on the host
    out = np.ascontiguousarray(
        res.results[0]["v"].reshape(B, D, NO).transpose(0, 2, 1)
    ).astype(np.float32, copy=False)
    if _trace:
        kernel.last_exec_time_ns = res.exec_time_ns
        kernel.last_results = res
    return out


# revision 26
# speedup vs baseline: 3.7135x; 1.0307x over previous
"""Trainium2 Bass kernel for CapsNet dynamic routing (nn_CapsRoutingLayer).

Reference computation:
    x_hat[b,i,o,d] = sum_k W[i,o,d,k] * x[b,i,k]
    b_logits = 0
    for it in 0..2:
        c = softmax_o(b_logits); s[b,o,d] = sum_i c[b,i,o] x_hat[b,i,o,d]
        v = squash(s)   # global Frobenius norm over the whole s tensor
        if it < 2: b_logits += sum_d x_hat[b,i,o,d] v[b,o,d]
    return v  # (128, 32, 32)

Sharding: input capsules i (1152) split across 8 cores (144 each); the tiny
[128,1024] per-iteration s is AllReduced. Per-core strategy:

  * All matmuls in bf16 (PE runs fp32 at 1/4 rate). W shard (9.4MB bf16)
    stays resident in SBUF, loaded in 4 chunks so pass-0 starts early.
  * Pass 0 (uniform c): s0 = (1/32) sum_i x_hat via K=128 matmuls.
  * Passes 1,2 regenerate x_hat per supergroup of 8 capsules into PSUM
    (2-capsule tiles, double-buffered), the Scalar engine copies it to SBUF
    as bf16, and all elementwise work runs on DVE in its 2x/4x 16-bit modes.
    Per-capsule s contributions are pair-summed in a small bf16 add tree on
    DVE; GpSimd (0.42 add efficiency) only accumulates one fp32 add per 16
    capsules.
  * squash's global norm is PSUM-free: tensor_tensor_reduce + GpSimd
    partition reduce/broadcast. Routing logits are linear in v, so pass 2
    uses m2 = v0 + v1 as its agreement multiplier (b2 = <x_hat, v0+v1>).

Software pipeline (emission order == per-engine execution order): supergroup
S's PE regen + ACT copies are emitted one round ahead of its softmax and s
accumulation, so no engine stalls on another's tail.
"""

import numpy as np
import ml_dtypes

from concourse import bacc, bass_isa, bass_utils, mybir, tile

N_CORES = 8
B = 128          # batch
NI = 1152        # input capsules
K = 32           # dim_input
NO = 32          # output capsules
D = 32           # dim_output
IC = NI // N_CORES   # input capsules per core = 144
NJ = IC // 4         # i-groups of 4 per core = 36
OD = NO * D          # 1024
SGC = 8              # capsules per supergroup
NSG = IC // SGC      # supergroups per pass = 18
NWC = 4              # W is DMA'd in NWC chunks of NJ//NWC j-groups
JC = NJ // NWC       # 9

F32 = mybir.dt.float32
BF16 = mybir.dt.bfloat16
ADD = mybir.AluOpType.add
MULT = mybir.AluOpType.mult
AXX = mybir.AxisListType.X
AXC = mybir.AxisListType.C
EXP = mybir.ActivationFunctionType.Exp

# Timing-ablation only: replace the cross-core AllReduce with a plain DMA
# (results become wrong; used to measure the collective's cost).
SKIP_COLLECTIVE = False
# Debug toggles for HW bisection.
USE_TTR = False     # tensor_tensor_reduce in squash
USE_STT = True


def _kernel_body(nc, tc, xs, ws, vout, repeats=1):
    with tc.tile_pool(name="persist", bufs=1) as per, \
         tc.tile_pool(name="xhp", bufs=3) as xhp, \
         tc.tile_pool(name="smallp", bufs=2) as smallp, \
         tc.tile_pool(name="pgp", bufs=2, space="PSUM") as pgp, \
         tc.tile_pool(name="dram", bufs=1, space="DRAM") as dram:

        W_t = [per.tile([128, JC, OD], BF16, name=f"W_t{c}") for c in range(NWC)]
        x_t = per.tile([128, NJ, 128], BF16)
        nc.sync.dma_start(x_t[:], xs[:])       # small; every matmul needs it
        for c in range(NWC):
            nc.sync.dma_start(W_t[c][:], ws[:, JC * c:JC * (c + 1), :])

        xv = per.tile([B, SGC * OD], BF16)     # agreement products (DVE-only)
        sx = per.tile([B, SGC * OD], BF16)     # c-weighted x_hat (DVE-only)
        s_acc = per.tile([B, OD], F32)         # local s accumulator
        s_loc = per.tile([B, OD], F32)         # pass-0 staging
        s_full = per.tile([B, OD], F32)        # post-AllReduce s
        v0 = per.tile([B, OD], F32)            # squash(s0)
        v_out = per.tile([B, OD], F32)         # final output
        m_bf = per.tile([B, OD], BF16)         # agreement multiplier (bf16)
        sq = per.tile([B, OD], F32)            # squash scratch
        col = per.tile([B, 1], F32)
        Sn1 = per.tile([1, 1], F32)
        Sb = per.tile([128, 1], F32)
        t1 = per.tile([128, 1], F32)
        t2 = per.tile([128, 1], F32)
        t3 = per.tile([128, 1], F32)
        gb = per.tile([128, 1], F32)
        ones128 = per.tile([128, 1], F32)
        ones1 = per.tile([1, 128], F32)
        nc.vector.memset(ones128[:], 1.0)
        nc.vector.memset(ones1[:], 1.0)

        ar_in = dram.tile([B, OD], F32)
        ar_out = dram.tile([B, OD], F32)

        def allreduce(src):
            nc.sync.dma_start(ar_in[:], src[:])
            if SKIP_COLLECTIVE:
                nc.sync.dma_start(ar_out[:], ar_in[:])
            else:
                nc.gpsimd.collective_compute(
                    "AllReduce", ADD,
                    replica_groups=[list(range(N_CORES))],
                    ins=[ar_in.opt()], outs=[ar_out.opt()],
                )
            nc.sync.dma_start(s_full[:], ar_out[:])

        def squash_mult(pass_idx):
            # g = sqrt(S)/(1+S) with S = global sum of squares of s_full,
            # then update the next pass's agreement multiplier / output.
            if USE_TTR:
                nc.vector.tensor_tensor_reduce(
                    out=sq[:], in0=s_full[:], in1=s_full[:], scale=1.0,
                    scalar=0.0, op0=MULT, op1=ADD, accum_out=col[:])
            else:
                nc.vector.tensor_mul(sq[:], s_full[:], s_full[:])
                nc.vector.tensor_reduce(out=col[:], in_=sq[:], axis=AXX,
                                        op=ADD)
            # partition reduce + broadcast via PE with ones (1-row matmuls)
            ps = pgp.tile([B, 2 * OD], F32, name="ps", tag="pg")
            nc.tensor.matmul(ps[0:1, 0:1], ones128[:], col[:],
                             start=True, stop=True)
            nc.vector.tensor_copy(Sn1[:], ps[0:1, 0:1])
            nc.tensor.matmul(ps[:, 512:513], ones1[:], Sn1[:],
                             start=True, stop=True)
            nc.vector.tensor_copy(Sb[:], ps[:, 512:513])
            nc.scalar.sqrt(t1[:], Sb[:])
            nc.vector.tensor_scalar_add(t2[:], Sb[:], 1.0)
            nc.vector.reciprocal(t3[:], t2[:])
            nc.vector.tensor_mul(gb[:], t1[:], t3[:])
            if pass_idx == 0:
                nc.vector.tensor_scalar_mul(v0[:], s_full[:], gb[:])
                nc.vector.tensor_copy(m_bf[:], v0[:])          # m1 = v0
            elif pass_idx == 1:
                if USE_STT:
                    nc.vector.scalar_tensor_tensor(             # m2 = g1*s1+v0
                        out=m_bf[:], in0=s_full[:], scalar=gb[:], in1=v0[:],
                        op0=MULT, op1=ADD)
                else:
                    nc.vector.tensor_scalar_mul(sq[:], s_full[:], gb[:])
                    nc.vector.tensor_add(m_bf[:], sq[:], v0[:])
            else:
                nc.vector.tensor_scalar_mul(v_out[:], s_full[:], gb[:])

        def regen_supergroup(S):
            # PE: x_hat for capsules 8S..8S+7 -> PSUM; ACT: cast-copy to SBUF
            xh = xhp.tile([B, SGC * OD], BF16, name="xh", tag="xh")
            for g4 in range(SGC // 2):
                pg = pgp.tile([B, 2 * OD], F32, name="pg", tag="pg")
                for slot in range(2):
                    i = SGC * S + 2 * g4 + slot
                    j, gg = divmod(i, 4)
                    wt = W_t[j // JC]
                    jj = j % JC
                    for h in range(2):
                        lo = slot * OD + 512 * h
                        nc.tensor.matmul(
                            pg[:, lo:lo + 512],
                            x_t[32 * gg:32 * (gg + 1), j, :],
                            wt[32 * gg:32 * (gg + 1), jj, 512 * h:512 * (h + 1)],
                            start=True, stop=True, tile_position=(32 * gg, 0))
                nc.scalar.copy(xh[:, 2 * OD * g4:2 * OD * (g4 + 1)], pg[:])
            return xh

        def run_pass(r):
            state = {}
            for S in range(NSG + 1):
                if S < NSG:
                    xh = regen_supergroup(S)
                    # agreement multiply: xv = xh * m (broadcast over capsule)
                    nc.vector.tensor_tensor(
                        out=xv[:].rearrange("b (i f) -> b i f", i=SGC),
                        in0=xh[:].rearrange("b (i f) -> b i f", i=SGC),
                        in1=m_bf[:].unsqueeze(1).broadcast_to([B, SGC, OD]),
                        op=MULT)
                if S >= 1:
                    st = state[S - 1]
                    nc.vector.tensor_reduce(
                        out=st["z"][:],
                        in_=st["e"][:].rearrange("b (i o) -> b i o", i=SGC),
                        axis=AXX, op=ADD)
                    nc.vector.reciprocal(st["rz"][:], st["z"][:])
                if S < NSG:
                    # reduce over d (the MIDDLE axis in the [b,i,d,o] layout,
                    # so every fold keeps a packed o innermost and runs in the
                    # DVE 2x mode; TensorReduce has no fast mode at all) as a
                    # log2 fold tree of adds.
                    a16 = smallp.tile([B, SGC * NO], BF16, name="a16", tag="a")
                    v4 = xv[:].rearrange("b (i d o) -> b i d o", i=SGC, d=D)
                    w = D // 2
                    while w > 1:
                        nc.vector.tensor_add(v4[:, :, 0:w, :], v4[:, :, 0:w, :],
                                             v4[:, :, w:2 * w, :])
                        w //= 2
                    nc.vector.tensor_add(
                        a16[:].rearrange("b (i o) -> b i o", i=SGC)
                            .unsqueeze(2),
                        v4[:, :, 0:1, :], v4[:, :, 1:2, :])
                    e16 = smallp.tile([B, SGC * NO], BF16, name="e16", tag="e")
                    nc.scalar.activation(e16[:], a16[:], EXP)
                    z16 = smallp.tile([B, SGC], F32, name="z16", tag="z")
                    rz16 = smallp.tile([B, SGC], F32, name="rz16", tag="rz")
                    state[S] = dict(e=e16, z=z16, rz=rz16, xh=xh)
                if S >= 1:
                    st = state[S - 1]
                    c16 = smallp.tile([B, SGC * NO], BF16, name="c16", tag="c")
                    nc.gpsimd.tensor_tensor(
                        out=c16[:].rearrange("b (i o) -> b i o", i=SGC),
                        in0=st["e"][:].rearrange("b (i o) -> b i o", i=SGC),
                        in1=st["rz"][:].unsqueeze(2).broadcast_to([B, SGC, NO]),
                        op=MULT)
                    # s contribution: sx = xh * c (broadcast over d; d is the
                    # middle axis so the innermost o stays packed -> DVE 2x)
                    nc.vector.tensor_tensor(
                        out=sx[:].rearrange("b (i d o) -> b i d o", i=SGC, d=D),
                        in0=st["xh"][:].rearrange("b (i d o) -> b i d o",
                                                  i=SGC, d=D),
                        in1=c16[:].rearrange("b (i o) -> b i o", i=SGC)
                            .unsqueeze(2).broadcast_to([B, SGC, D, NO]),
                        op=MULT)
                    # capsule-sum tree, split DVE (blocks 0-3) / GpSimd (4-7);
                    # GpSimd then owns the serial fp32 s_acc accumulation, so
                    # DVE never waits on the slower engine.
                    nc.vector.tensor_add(sx[:, 0:OD], sx[:, 0:OD],
                                         sx[:, OD:2 * OD])
                    nc.vector.tensor_add(sx[:, 2 * OD:3 * OD],
                                         sx[:, 2 * OD:3 * OD],
                                         sx[:, 3 * OD:4 * OD])
                    nc.vector.tensor_add(sx[:, 0:OD], sx[:, 0:OD],
                                         sx[:, 2 * OD:3 * OD])
                    nc.gpsimd.tensor_add(sx[:, 4 * OD:5 * OD],
                                         sx[:, 4 * OD:5 * OD],
                                         sx[:, 5 * OD:6 * OD])
                    nc.gpsimd.tensor_add(sx[:, 6 * OD:7 * OD],
                                         sx[:, 6 * OD:7 * OD],
                                         sx[:, 7 * OD:8 * OD])
                    nc.gpsimd.tensor_add(sx[:, 4 * OD:5 * OD],
                                         sx[:, 4 * OD:5 * OD],
                                         sx[:, 6 * OD:7 * OD])
                    if S - 1 == 0:
                        nc.gpsimd.tensor_add(s_acc[:], sx[:, 0:OD],
                                             sx[:, 4 * OD:5 * OD])
                    else:
                        nc.gpsimd.tensor_add(s_acc[:], s_acc[:], sx[:, 0:OD])
                        nc.gpsimd.tensor_add(s_acc[:], s_acc[:],
                                             sx[:, 4 * OD:5 * OD])
            allreduce(s_acc)
            squash_mult(r)

        with nc.allow_low_precision("bf16 routing; tolerance is 2e-2"):
            for _rep in range(repeats):
                # ---- pass 0: s0 = (1/32) sum_i x_hat, K=128 matmuls
                pg0 = pgp.tile([B, 2 * OD], F32, name="pg0", tag="pg")
                for j in range(NJ):
                    wt = W_t[j // JC]
                    jj = j % JC
                    for h in range(2):
                        nc.tensor.matmul(
                            pg0[:, 512 * h:512 * (h + 1)],
                            x_t[:, j, :], wt[:, jj, 512 * h:512 * (h + 1)],
                            start=(j == 0), stop=(j == NJ - 1))
                nc.vector.tensor_scalar_mul(s_loc[:], pg0[:, 0:OD], 1.0 / NO)
                allreduce(s_loc)
                squash_mult(0)
                run_pass(1)
                run_pass(2)

        nc.sync.dma_start(vout[:], v_out[:])


_NC_CACHE = {}


def _build(repeats=1):
    if repeats in _NC_CACHE:
        return _NC_CACHE[repeats]
    nc = bacc.Bacc("TRN2", target_bir_lowering=False, debug=False,
                   num_devices=N_CORES)
    xs = nc.dram_tensor("xs", [128, NJ, 128], BF16, kind="ExternalInput").ap()
    ws = nc.dram_tensor("ws", [128, NJ, OD], BF16, kind="ExternalInput").ap()
    vout = nc.dram_tensor("v", [B, OD], F32, kind="ExternalOutput").ap()
    with tile.TileContext(nc) as tc:
        _kernel_body(nc, tc, xs, ws, vout, repeats=repeats)
    nc.compile()
    _NC_CACHE[repeats] = nc
    return nc


def _shard_inputs(x, W):
    BF = ml_dtypes.bfloat16
    in_maps = []
    for c in range(N_CORES):
        i0 = c * IC
        wc = W[i0:i0 + IC]                          # (144, 32, 32, 32) iodk
        # (d,o)-transposed columns: ws[(g,k), j, (d,o)] = W[i0+4j+g, o, d, k]
        wsn = np.ascontiguousarray(
            wc.reshape(NJ, 4, NO, D, K).transpose(1, 4, 0, 3, 2)
              .reshape(128, NJ, OD)).astype(BF)
        xc = x[:, i0:i0 + IC, :]                    # (128, 144, 32) bik
        xt = np.ascontiguousarray(
            xc.reshape(B, NJ, 4, K).transpose(2, 3, 1, 0)
              .reshape(128, NJ, 128)).astype(BF)
        in_maps.append({"xs": xt, "ws": wsn})
    return in_maps


def kernel(x, W, _trace=False):
    x = np.asarray(x, dtype=np.float32)
    W = np.asarray(W, dtype=np.float32)
    nc = _build()
    in_maps = _shard_inputs(x, W)
    res = bass_utils.run_bass_kernel_spmd(
        nc, in_maps, core_ids=list(range(N_CORES)), trace=_trace)
    # kernel works in (d,o)-transposed layout; untranspose on the host
    out = np.ascontiguousarray(
        res.results[0]["v"].reshape(B, D, NO).transpose(0, 2, 1)
    ).astype(np.float32, copy=False)
    if _trace:
        kernel.last_exec_time_ns = res.exec_time_ns
        kernel.last_results = res
    return out
